# revision 26
# baseline (speedup 1.0000x reference)
"""AnchorOnlyMixtureRNN — 8-core Trainium2 kernel.

Architecture (scatter_memory): the model is two sequential scans plus dense
ops. The dominant cost — the 1024-step anchor-value (av) gated-LayerNorm
recurrence over state [B, A=64, D=512] — runs on the 8 NeuronCores, batch-
sharded 4 batches/core (pure data parallelism: the scan state is per-batch).
The cheap/BLAS-friendly parts (embedding gather, z-trajectory, collapsed
gate GEMM, 256-step decoder, vocab logits, log-softmax) run on host.

Device kernel per core (B_local=4):
  chains (b, a) -> tile s = b//2, partition p = a + 64*(b%2)
  - Z ships int8 (scale 1/16; dequant folded into the E selection matmul),
    staged from DRAM in 64-step blocks via SWDGE cast-DMA to [4, 64, 512].
  - per step: zb_s = E_s.T @ Zstage[:, j, :] on TensorE -> PSUM;
    state kept as u = x/g so the blend is u = (u_prev - MP)*C' + zb
    (one tensor_scalar + one tensor_tensor on VectorE per tile);
    sum(x) is analytic (host-shipped tables), sumsq via ScalarE Square
    with accum; rstd = 1/std via two Newton iterations seeded from the
    previous step (exact ACT sqrt for the first 8 transient steps, h
    clamped on the first iteration to survive variance jumps).
  - per-step coefficient tables (MP, GSQ, V2, GC) are host-precomputed,
    delta-encoded around their constant centers so bf16 keeps the tiny
    anchor-differentiating gate signal, and cast to f32 on device once.
  - final normalize + bf16 DMA out.

Walrus in this toolchain allows at most ONE sync wait per instruction:
the kernel must be built with bacc.Bacc (its finalize pipeline splits
waits via event semaphores); plain bass.Bass modules fail codegen.
tensor_tensor_reduce crashes the device at runtime — avoided.
"""
import math
import numpy as np

D = 512
A = 64
V_OUT = 32000
B = 32
S_ENC = 1024
S_DEC = 256
EPS = 1e-6
N_CORES = 8
BL = B // N_CORES          # 4 local batches per core
TBLK = 64                  # z staging block (steps)
QS = 16.0                  # int8 quant scale for Z
KG = 8000.0                # int8 quant scale for (g - 0.5)

_CACHE = {}


def _ln(x, g, b):
    m = x.mean(axis=-1, keepdims=True)
    s = x.std(axis=-1, ddof=1, keepdims=True)
    return g * (x - m) / (s + EPS) + b


def _sigmoid(x):
    return 1.0 / (1.0 + np.exp(-x))


# ---------------------------------------------------------------- Bass ----
def _build_phase3_nc(S=S_ENC):
    import concourse.bacc as bacc
    import concourse.tile as tile
    from concourse import mybir

    f32 = mybir.dt.float32
    bf16 = mybir.dt.bfloat16
    i8 = mybir.dt.int8
    Alu = mybir.AluOpType
    Act = mybir.ActivationFunctionType

    from concourse.tile_rust import add_dep_helper

    nc = bacc.Bacc("TRN2", target_bir_lowering=False)
    Z_d = nc.declare_dram_parameter("Z", [BL, S, D], i8, isOutput=False)
    E_d = nc.declare_dram_parameter("E", [BL, 2, 128], bf16, isOutput=False)
    # per-step coefficient tables (host-precomputed, see _pack_inputs):
    #   MP[.,.,t]  = zsum_t/512            (= m_t/g_t)
    #   GSQ[.,.,t] = g_t^2
    #   V2[.,.,t]  = 512*m_t^2
    #   GC[.,.,t]  = g_t*(1-g_{t+1})/g_{t+1}
    MP_d = nc.declare_dram_parameter("MP", [128, 2, S], bf16, isOutput=False)
    GSQ_d = nc.declare_dram_parameter("GSQ", [128, 2, S], bf16, isOutput=False)
    V2_d = nc.declare_dram_parameter("V2", [128, 2, S], bf16, isOutput=False)
    GC_d = nc.declare_dram_parameter("GC", [128, 2, S], bf16, isOutput=False)
    GFIN_d = nc.declare_dram_parameter("GFIN", [128, 2], f32, isOutput=False)
    out_d = nc.declare_dram_parameter("avout", [2, 128, D], bf16, isOutput=True)

    with tile.TileContext(nc) as tc:
        with (
            tc.tile_pool(name="big", bufs=1) as big,
            tc.tile_pool(name="stage", bufs=2) as stg,
            tc.tile_pool(name="work", bufs=2) as work,
            tc.tile_pool(name="st", bufs=2) as stp,
            tc.tile_pool(name="ps", bufs=4, space="PSUM") as ps,
        ):
            Et = big.tile([BL, 2, 128], bf16, tag="E")
            nc.sync.dma_start(Et[:], E_d[:])
            GFINt = big.tile([128, 2], f32, tag="GFIN")
            nc.sync.dma_start(GFINt[:], GFIN_d[:])
            # load coefficient tables, cast bf16 -> f32 (scalar-AP operands)
            tabs = {}
            centers = {"MP": None, "GSQ": 0.25 / 511.0, "V2": None,
                       "GC": 0.5}
            for nm, dram in (("MP", MP_d), ("GSQ", GSQ_d), ("V2", V2_d),
                             ("GC", GC_d)):
                tb = big.tile([128, 2, S], bf16, name=f"{nm}b", tag=f"{nm}b")
                nc.sync.dma_start(tb[:], dram[:])
                tf = big.tile([128, 2, S], f32, name=f"{nm}f", tag=f"{nm}f")
                if centers[nm] is None:
                    nc.vector.tensor_copy(tf[:], tb[:])
                else:
                    nc.vector.tensor_scalar(tf[:], tb[:], centers[nm], None,
                                            Alu.add)
                tabs[nm] = tf
            MPt, GSQt, V2t, GCt = (tabs[k] for k in ("MP", "GSQ", "V2", "GC"))

            # persistent state: u = x/g per tile, double-buffered so the
            # scalar-engine Square read of step t doesn't block step t+1's
            # state write (WAR)
            u = [[big.tile([128, D], f32, name=f"us{s}{k}", tag=f"us{s}{k}")
                  for k in range(2)] for s in range(2)]
            Cp = [None, None]      # per-tile blend coefficient
            rstd_p = [None, None]  # per-tile rstd (Newton seed)

            def emit_stats(t, s, s2, after=None):
                # var' = var/511 = s2u*gsq' - v2' ; rstd = 1/sqrt(var')
                var = stp.tile([128, 1], f32, name=f"var{s}", tag=f"var{s}")
                vi = nc.vector.tensor_scalar(
                    var[:], s2[:], GSQt[:, s, t:t + 1],
                    V2t[:, s, t:t + 1], Alu.mult, Alu.subtract)
                del vi
                rstd = stp.tile([128, 1], f32, name=f"rstd{s}",
                                tag=f"rstd{s}")
                if t < 8:
                    std = stp.tile([128, 1], f32, name=f"std{s}",
                                   tag=f"std{s}")
                    nc.scalar.activation(std[:], var[:], Act.Sqrt)
                    stde = stp.tile([128, 1], f32, name=f"stde{s}",
                                    tag=f"stde{s}")
                    nc.vector.tensor_scalar(stde[:], std[:], EPS, None,
                                            Alu.add)
                    nc.vector.reciprocal(rstd[:], stde[:])
                else:
                    # one clamped Newton rsqrt iteration from prev rstd
                    r = rstd_p[s]
                    r2 = stp.tile([128, 1], f32, name=f"nr2{s}",
                                  tag=f"nr2{s}")
                    nc.vector.tensor_tensor(r2[:], r[:], r[:], Alu.mult)
                    w = stp.tile([128, 1], f32, name=f"nw{s}", tag=f"nw{s}")
                    nc.vector.tensor_tensor(w[:], var[:], r2[:], Alu.mult)
                    h = stp.tile([128, 1], f32, name=f"nh{s}", tag=f"nh{s}")
                    nc.vector.tensor_scalar(h[:], w[:], -0.5, 1.5,
                                            Alu.mult, Alu.add)
                    nc.vector.tensor_scalar(h[:], h[:], 0.25, None, Alu.max)
                    nc.vector.tensor_tensor(rstd[:], r[:], h[:], Alu.mult)
                Cn = stp.tile([128, 1], f32, name=f"Cn{s}", tag=f"Cn{s}")
                if t + 1 < S:
                    nc.vector.tensor_tensor(Cn[:], GCt[:, s, t:t + 1],
                                            rstd[:], Alu.mult)
                else:
                    nc.vector.tensor_tensor(Cn[:], GFINt[:, s:s + 1],
                                            rstd[:], Alu.mult)
                Cp[s] = Cn
                rstd_p[s] = rstd

            pending = None
            nblk = (S + TBLK - 1) // TBLK
            for blk in range(nblk):
                t0b = blk * TBLK
                nstep = min(TBLK, S - t0b)
                zst = stg.tile([BL, TBLK, D], bf16, tag="zst")
                nc.gpsimd.dma_start(zst[:, :nstep, :],
                                    Z_d[:, t0b:t0b + nstep, :])
                for j in range(nstep):
                    t = t0b + j
                    for s in range(2):
                        zbs = ps.tile([128, D], f32, name=f"zb{s}",
                                      tag=f"zb{s}")
                        nc.tensor.matmul(zbs[:], Et[:, s, :], zst[:, j, :],
                                         start=True, stop=True)
                        ucur = u[s][t % 2]
                        if t == 0:
                            ui = nc.vector.tensor_copy(ucur[:], zbs[:])
                        else:
                            # y' = (u_prev - MP_{t-1}) * C' ; u = y' + zb
                            y = work.tile([128, D], f32, name=f"y{s}",
                                          tag=f"y{s}")
                            nc.vector.tensor_scalar(
                                y[:], u[s][(t - 1) % 2][:],
                                MPt[:, s, t - 1:t],
                                Cp[s][:], Alu.subtract, Alu.mult)
                            ui = nc.vector.tensor_tensor(ucur[:], y[:],
                                                         zbs[:], Alu.add)
                        s2 = stp.tile([128, 1], f32, name=f"s2{s}",
                                      tag=f"s2{s}")
                        usq = work.tile([128, D], f32, name=f"usq{s}",
                                        tag=f"usq{s}")
                        nc.scalar.activation(usq[:], ucur[:], Act.Square,
                                             accum_out=s2[:])
                        # emit the PREVIOUS half-step's stats now: its ACT
                        # dependency is already satisfied, so the in-order
                        # DVE stream never stalls on the scalar engine
                        del ui
                        if pending is not None:
                            emit_stats(*pending)
                        pending = (t, s, s2)
            emit_stats(*pending)

            # final normalize: av = (u - MP_{S-1}) * (g_{S-1}*rstd), DMA out
            for s in range(2):
                avf = work.tile([128, D], bf16, name=f"avf{s}", tag=f"avf{s}")
                nc.vector.tensor_scalar(
                    avf[:], u[s][(S - 1) % 2][:], MPt[:, s, S - 1:S],
                    Cp[s][:], Alu.subtract, Alu.mult)
                nc.sync.dma_start(out_d[s], avf[:])
    nc.finalize()
    return nc


def _make_E():
    E = np.zeros((BL, 2, 128), np.float32)
    for s in range(2):
        for h in range(2):
            E[2 * s + h, s, 64 * h:64 * (h + 1)] = 1.0 / QS
    return E


def _get_nc():
    if "nc" not in _CACHE:
        _CACHE["nc"] = _build_phase3_nc()
    return _CACHE["nc"]


def _pack128(a):
    """[S, 4, A] -> [128, 2, S] with p = anchor + 64*(b%2), s = b//2."""
    return np.ascontiguousarray(
        a.reshape(S_ENC, 2, 2, A).transpose(2, 3, 1, 0)       # [h, a, s, t]
    ).reshape(128, 2, S_ENC)


def _pack_inputs(Z, G_all):
    """Z [S,B,D] f32, G_all [S,B,A] f32 -> per-core in_maps."""
    import ml_dtypes
    bf16 = ml_dtypes.bfloat16
    f = np.float32
    Zq = np.clip(np.rint(Z * QS), -127, 127).astype(np.int8)  # [S,B,D]
    zsum = Zq.astype(f).sum(axis=2) / QS                      # [S,B]
    E = _make_E().astype(bf16)
    in_maps = []
    for i in range(N_CORES):
        bs = slice(4 * i, 4 * i + 4)
        zc = np.ascontiguousarray(Zq[:, bs, :].transpose(1, 0, 2))  # [BL,S,D]
        g = G_all[:, bs, :].astype(f)                         # [S,4,A]
        zs = zsum[:, bs]                                      # [S,4]
        m = g * (zs / D)[:, :, None]                          # m_t
        mp = np.broadcast_to((zs / D)[:, :, None], g.shape)   # m/g
        gsq = (g * g - 0.25) / (D - 1)
        v2 = D * m * m / (D - 1)
        gc = np.zeros_like(g)
        gc[:-1] = g[:-1] * (1.0 - g[1:]) / g[1:] - 0.5
        gfin = _pack128(np.broadcast_to(g[-1:], g.shape))[:, :, 0].astype(f)
        in_maps.append({
            "Z": zc, "E": E,
            "MP": _pack128(mp).astype(bf16),
            "GSQ": _pack128(gsq).astype(bf16),
            "V2": _pack128(v2).astype(bf16),
            "GC": _pack128(gc).astype(bf16),
            "GFIN": np.ascontiguousarray(gfin),
        })
    return in_maps


def _unpack_av(results):
    av = np.empty((B, A, D), np.float32)
    for i in range(N_CORES):
        o = np.asarray(results[i]["avout"], np.float32)   # [2, 128, D]
        for s in range(2):
            for h in range(2):
                av[4 * i + 2 * s + h] = o[s, 64 * h:64 * (h + 1), :]
    return av


def _phase3_on_trn(Z, G_all):
    from concourse.bass_utils import run_bass_kernel_spmd
    nc = _get_nc()
    in_maps = _pack_inputs(Z, G_all)
    for m in in_maps:
        for k in ("MP", "GSQ", "V2", "GC", "GFIN"):
            if not np.isfinite(np.asarray(m[k], np.float32)).all():
                raise ValueError("non-finite coefficient table")
    res = run_bass_kernel_spmd(nc, in_maps, core_ids=list(range(N_CORES)))
    av = _unpack_av(res.results)
    if not np.isfinite(av).all():
        raise ValueError("non-finite device output")
    return av


def _phase3_host(Z, G_all, n1_g, n1_b):
    """Fallback: vectorized numpy recurrence."""
    f = np.float32
    av = np.zeros((B, A, D), f)
    X = np.empty((B, A, D), f)
    for t in range(S_ENC):
        g = G_all[t][:, :, None]
        np.subtract(Z[t][:, None, :], av, out=X)
        X *= g
        av += X
        m = av.mean(-1, keepdims=True)
        av -= m
        q = np.einsum('bad,bad->ba', av, av)
        s = np.sqrt(q / (D - 1)) + EPS
        av /= s[:, :, None]
        if n1_g is not None:
            av *= n1_g
            av += n1_b
    return av


# --------------------------------------------------------------- model ----
def kernel(input_sequence, output_sequence, emb_in, emb_out, enc_key_W,
           enc_Wq, enc_bq, enc_Wk, enc_bk, n1_g, n1_b, dec_key_W,
           rdr_Wq, rdr_bq, rdr_Wk, rdr_bk, rdr_Wv, rdr_bv,
           dat_Wq, dat_bq, dat_Wk, dat_bk, n2_g, n2_b, n3_g, n3_b,
           voc_W, voc_b):
    f = np.float32
    emb_in = np.asarray(emb_in, f)
    scale = f(1.0 / math.sqrt(D))
    sqrtD = f(math.sqrt(D))
    idx = np.asarray(input_sequence)
    n1_g = np.asarray(n1_g, f)
    n1_b = np.asarray(n1_b, f)

    # -- encoder phase 1: z-trajectory (independent of av) --
    x_enc = emb_in[idx] * sqrtD                                # [B,S,D]
    Z = np.empty((S_ENC, B, D), f)
    z = np.zeros((B, D), f)
    for t in range(S_ENC):
        z = z + x_enc[:, t]
        m = z.mean(-1, keepdims=True)
        z -= m
        q = np.einsum('bd,bd->b', z, z)
        sd = np.sqrt(q / (D - 1)) + EPS
        z /= sd[:, None]
        if n1_g is not None:
            z *= n1_g
            z += n1_b
        Z[t] = z

    # -- encoder phase 2: batched gates (collapsed GEMM) --
    Qa = enc_key_W @ enc_Wq.T + enc_bq                         # [A,D]
    W2 = (enc_Wk.T @ Qa.T).astype(f)                           # [D,A]
    c2 = (enc_bk @ Qa.T).astype(f)                             # [A]
    G_all = _sigmoid((Z.reshape(-1, D) @ W2 + c2) * scale).reshape(
        S_ENC, B, A)

    # -- encoder phase 3: anchor-value recurrence on the NeuronCores --
    # device kernel computes plain LN; apply affine n1_g/n1_b after if
    # they are not identity (setup uses ones/zeros).
    affine = not (np.allclose(n1_g, 1.0) and np.allclose(n1_b, 0.0))
    if affine:
        av = _phase3_host(Z, G_all, n1_g, n1_b)
    else:
        try:
            av = _phase3_on_trn(Z, G_all)
        except Exception:
            av = _phase3_host(Z, G_all, None, None)

    # -- decoder (avx carry is dead code; z path only) --
    Kr = av @ rdr_Wk.T + rdr_bk                                # [B,A,D]
    Vr = av @ rdr_Wv.T + rdr_bv
    # fold the q-projection into the attention: s = zd @ M[b] + c[b]
    M = np.einsum('ed,bae->bda', np.asarray(rdr_Wq, f), Kr)    # [B,D,A]
    c = np.einsum('e,bae->ba', np.asarray(rdr_bq, f), Kr)      # [B,A]
    n2_g = np.asarray(n2_g, f)
    n2_b = np.asarray(n2_b, f)
    zd = Z[-1].copy()                                          # [B,D]
    for t in range(S_DEC):
        a = (np.einsum('bd,bda->ba', zd, M) + c) * scale       # [B,A]
        a -= a.max(axis=-1, keepdims=True)
        e = np.exp(a)
        e /= e.sum(axis=-1, keepdims=True)
        zd = zd + np.einsum('ba,bad->bd', e, Vr)
        m = zd.mean(-1, keepdims=True)
        zd -= m
        q = np.einsum('bd,bd->b', zd, zd)
        sd = np.sqrt(q / (D - 1)) + EPS
        zd /= sd[:, None]
        zd *= n2_g
        zd += n2_b

    # -- logits + log_softmax on host --
    zfin = zd.astype(f)                                        # [B,D]
    logits = zfin @ np.asarray(voc_W, f).T + voc_b             # [B,V]
    logits = logits[:, None, :]
    mx = logits.max(axis=-1, keepdims=True)
    lse = np.log(np.exp(logits - mx).sum(axis=-1, keepdims=True)) + mx
    return (logits - lse).astype(f)


# ------------------------------------------------------------- profile ----
def _profile():
    """Best-available timing of the bass kernel: HW NTFF if possible,
    else CoreSim cost-model time. Returns (exec_ns, source)."""
    nc = _get_nc()
    rng = np.random.default_rng(0)
    Z = rng.standard_normal((S_ENC, B, D)).astype(np.float32)
    G = (0.5 + 0.01 * rng.standard_normal((S_ENC, B, A))).astype(np.float32)
    in_maps = _pack_inputs(Z, G)
    try:
        from concourse.bass_utils import run_bass_kernel_spmd
        res = run_bass_kernel_spmd(nc, in_maps,
                                   core_ids=list(range(N_CORES)), trace=True)
        if res.exec_time_ns:
            return res.exec_time_ns, "hw-ntff"
    except Exception:
        pass
    from concourse.bass_interp import CoreSim
    sim = CoreSim(nc, publish_trace=False)
    for k, v in in_maps[0].items():
        sim.tensor(k)[:] = v
    sim.simulate()
    return int(sim.time), "coresim"


# revision 27
# speedup vs baseline: 1.0794x; 1.0794x over previous
"""AnchorOnlyMixtureRNN — 8-core Trainium2 kernel.

Architecture (scatter_memory): the model is two sequential scans plus dense
ops. The dominant cost — the 1024-step anchor-value (av) gated-LayerNorm
recurrence over state [B, A=64, D=512] — runs on the 8 NeuronCores, batch-
sharded 4 batches/core (pure data parallelism: the scan state is per-batch).
The cheap/BLAS-friendly parts (embedding gather, z-trajectory, collapsed
gate GEMM, 256-step decoder, vocab logits, log-softmax) run on host.

Device kernel per core (B_local=4):
  chains (b, a) -> tile s = b//2, partition p = a + 64*(b%2)
  - Z ships int8 (scale 1/16; dequant folded into the E selection matmul),
    staged from DRAM in 64-step blocks via SWDGE cast-DMA to [4, 64, 512].
  - per step: zb_s = E_s.T @ Zstage[:, j, :] on TensorE -> PSUM;
    state kept as u = x/g so the blend is u = (u_prev - MP)*C' + zb
    (one tensor_scalar + one tensor_tensor on VectorE per tile);
    sum(x) is analytic (host-shipped tables), sumsq via ScalarE Square
    with accum; rstd = 1/std via two Newton iterations seeded from the
    previous step (exact ACT sqrt for the first 8 transient steps, h
    clamped on the first iteration to survive variance jumps).
  - per-step coefficient tables (MP, GSQ, V2, GC) are host-precomputed,
    delta-encoded around their constant centers so bf16 keeps the tiny
    anchor-differentiating gate signal, and cast to f32 on device once.
  - final normalize + bf16 DMA out.

Walrus in this toolchain allows at most ONE sync wait per instruction:
the kernel must be built with bacc.Bacc (its finalize pipeline splits
waits via event semaphores); plain bass.Bass modules fail codegen.
tensor_tensor_reduce crashes the device at runtime — avoided.
"""
import math
import numpy as np

D = 512
A = 64
V_OUT = 32000
B = 32
S_ENC = 1024
S_DEC = 256
EPS = 1e-6
N_CORES = 8
BL = B // N_CORES          # 4 local batches per core
TBLK = 64                  # z staging block (steps)
QS = 16.0                  # int8 quant scale for Z
KG = 8000.0                # int8 quant scale for (g - 0.5)

_CACHE = {}


def _ln(x, g, b):
    m = x.mean(axis=-1, keepdims=True)
    s = x.std(axis=-1, ddof=1, keepdims=True)
    return g * (x - m) / (s + EPS) + b


def _sigmoid(x):
    return 1.0 / (1.0 + np.exp(-x))


# ---------------------------------------------------------------- Bass ----
def _build_phase3_nc(S=S_ENC):
    import concourse.bacc as bacc
    import concourse.tile as tile
    from concourse import mybir

    f32 = mybir.dt.float32
    bf16 = mybir.dt.bfloat16
    i8 = mybir.dt.int8
    Alu = mybir.AluOpType
    Act = mybir.ActivationFunctionType

    from concourse.tile_rust import add_dep_helper

    nc = bacc.Bacc("TRN2", target_bir_lowering=False)
    Z_d = nc.declare_dram_parameter("Z", [BL, S, D], i8, isOutput=False)
    E_d = nc.declare_dram_parameter("E", [BL, 2, 128], bf16, isOutput=False)
    # per-step coefficient tables (host-precomputed, see _pack_inputs):
    #   MP[.,.,t]  = zsum_t/512            (= m_t/g_t)
    #   GSQ[.,.,t] = g_t^2
    #   V2[.,.,t]  = 512*m_t^2
    #   GC[.,.,t]  = g_t*(1-g_{t+1})/g_{t+1}
    MP_d = nc.declare_dram_parameter("MP", [128, 2, S], bf16, isOutput=False)
    GSQ_d = nc.declare_dram_parameter("GSQ", [128, 2, S], bf16, isOutput=False)
    V2_d = nc.declare_dram_parameter("V2", [128, 2, S], bf16, isOutput=False)
    GC_d = nc.declare_dram_parameter("GC", [128, 2, S], bf16, isOutput=False)
    GFIN_d = nc.declare_dram_parameter("GFIN", [128, 2], f32, isOutput=False)
    out_d = nc.declare_dram_parameter("avout", [2, 128, D], bf16, isOutput=True)

    with tile.TileContext(nc) as tc:
        with (
            tc.tile_pool(name="big", bufs=1) as big,
            tc.tile_pool(name="stage", bufs=2) as stg,
            tc.tile_pool(name="work", bufs=2) as work,
            tc.tile_pool(name="st", bufs=2) as stp,
            tc.tile_pool(name="ps", bufs=4, space="PSUM") as ps,
        ):
            Et = big.tile([BL, 2, 128], bf16, tag="E")
            nc.sync.dma_start(Et[:], E_d[:])
            GFINt = big.tile([128, 2], f32, tag="GFIN")
            nc.sync.dma_start(GFINt[:], GFIN_d[:])
            # load coefficient tables, cast bf16 -> f32 (scalar-AP operands)
            tabs = {}
            centers = {"MP": None, "GSQ": 0.25 / 511.0, "V2": None,
                       "GC": 0.5}
            for nm, dram in (("MP", MP_d), ("GSQ", GSQ_d), ("V2", V2_d),
                             ("GC", GC_d)):
                tb = big.tile([128, 2, S], bf16, name=f"{nm}b", tag=f"{nm}b")
                nc.sync.dma_start(tb[:], dram[:])
                tf = big.tile([128, 2, S], f32, name=f"{nm}f", tag=f"{nm}f")
                if centers[nm] is None:
                    nc.vector.tensor_copy(tf[:], tb[:])
                else:
                    nc.vector.tensor_scalar(tf[:], tb[:], centers[nm], None,
                                            Alu.add)
                tabs[nm] = tf
            MPt, GSQt, V2t, GCt = (tabs[k] for k in ("MP", "GSQ", "V2", "GC"))

            # persistent state: u = x/g per tile, double-buffered so the
            # scalar-engine Square read of step t doesn't block step t+1's
            # state write (WAR)
            u = [[big.tile([128, D], f32, name=f"us{s}{k}", tag=f"us{s}{k}")
                  for k in range(2)] for s in range(2)]
            Cp = [None, None]      # per-tile blend coefficient
            rstd_p = [None, None]  # per-tile rstd (Newton seed)

            def emit_stats(t, s, s2, after=None):
                # stats chain on the (otherwise idle) GpSimd engine so its
                # ACT-wait never stalls the in-order DVE stream
                var = stp.tile([128, 1], f32, name=f"var{s}", tag=f"var{s}")
                nc.gpsimd.tensor_scalar(
                    var[:], s2[:], GSQt[:, s, t:t + 1],
                    V2t[:, s, t:t + 1], Alu.mult, Alu.subtract)
                rstd = stp.tile([128, 1], f32, name=f"rstd{s}",
                                tag=f"rstd{s}")
                if t < 8:
                    std = stp.tile([128, 1], f32, name=f"std{s}",
                                   tag=f"std{s}")
                    nc.scalar.activation(std[:], var[:], Act.Sqrt)
                    stde = stp.tile([128, 1], f32, name=f"stde{s}",
                                    tag=f"stde{s}")
                    nc.vector.tensor_scalar(stde[:], std[:], EPS, None,
                                            Alu.add)
                    nc.vector.reciprocal(rstd[:], stde[:])
                else:
                    # one clamped Newton rsqrt iteration from prev rstd
                    r = rstd_p[s]
                    r2 = stp.tile([128, 1], f32, name=f"nr2{s}",
                                  tag=f"nr2{s}")
                    nc.gpsimd.tensor_tensor(r2[:], r[:], r[:], Alu.mult)
                    w = stp.tile([128, 1], f32, name=f"nw{s}", tag=f"nw{s}")
                    nc.gpsimd.tensor_tensor(w[:], var[:], r2[:], Alu.mult)
                    h = stp.tile([128, 1], f32, name=f"nh{s}", tag=f"nh{s}")
                    nc.gpsimd.tensor_scalar(h[:], w[:], -0.5, 1.5,
                                            Alu.mult, Alu.add)
                    nc.gpsimd.tensor_scalar(h[:], h[:], 0.25, None, Alu.max)
                    nc.gpsimd.tensor_tensor(rstd[:], r[:], h[:], Alu.mult)
                Cn = stp.tile([128, 1], f32, name=f"Cn{s}", tag=f"Cn{s}")
                if t + 1 < S:
                    nc.gpsimd.tensor_tensor(Cn[:], GCt[:, s, t:t + 1],
                                            rstd[:], Alu.mult)
                else:
                    nc.gpsimd.tensor_tensor(Cn[:], GFINt[:, s:s + 1],
                                            rstd[:], Alu.mult)
                Cp[s] = Cn
                rstd_p[s] = rstd

            pending = None
            nblk = (S + TBLK - 1) // TBLK
            for blk in range(nblk):
                t0b = blk * TBLK
                nstep = min(TBLK, S - t0b)
                zst = stg.tile([BL, TBLK, D], bf16, tag="zst")
                nc.gpsimd.dma_start(zst[:, :nstep, :],
                                    Z_d[:, t0b:t0b + nstep, :])
                for j in range(nstep):
                    t = t0b + j
                    for s in range(2):
                        zbs = ps.tile([128, D], f32, name=f"zb{s}",
                                      tag=f"zb{s}")
                        nc.tensor.matmul(zbs[:], Et[:, s, :], zst[:, j, :],
                                         start=True, stop=True)
                        ucur = u[s][t % 2]
                        if t == 0:
                            ui = nc.vector.tensor_copy(ucur[:], zbs[:])
                        else:
                            # y' = (u_prev - MP_{t-1}) * C' ; u = y' + zb
                            y = work.tile([128, D], f32, name=f"y{s}",
                                          tag=f"y{s}")
                            nc.vector.tensor_scalar(
                                y[:], u[s][(t - 1) % 2][:],
                                MPt[:, s, t - 1:t],
                                Cp[s][:], Alu.subtract, Alu.mult)
                            ui = nc.vector.tensor_tensor(ucur[:], y[:],
                                                         zbs[:], Alu.add)
                        s2 = stp.tile([128, 1], f32, name=f"s2{s}",
                                      tag=f"s2{s}")
                        usq = work.tile([128, D], f32, name=f"usq{s}",
                                        tag=f"usq{s}")
                        nc.scalar.activation(usq[:], ucur[:], Act.Square,
                                             accum_out=s2[:])
                        # emit the PREVIOUS half-step's stats now: its ACT
                        # dependency is already satisfied, so the in-order
                        # DVE stream never stalls on the scalar engine
                        del ui
                        if pending is not None:
                            emit_stats(*pending)
                        pending = (t, s, s2)
            emit_stats(*pending)

            # final normalize: av = (u - MP_{S-1}) * (g_{S-1}*rstd), DMA out
            for s in range(2):
                avf = work.tile([128, D], bf16, name=f"avf{s}", tag=f"avf{s}")
                nc.vector.tensor_scalar(
                    avf[:], u[s][(S - 1) % 2][:], MPt[:, s, S - 1:S],
                    Cp[s][:], Alu.subtract, Alu.mult)
                nc.sync.dma_start(out_d[s], avf[:])
    nc.finalize()
    return nc


def _make_E():
    E = np.zeros((BL, 2, 128), np.float32)
    for s in range(2):
        for h in range(2):
            E[2 * s + h, s, 64 * h:64 * (h + 1)] = 1.0 / QS
    return E


def _get_nc():
    if "nc" not in _CACHE:
        _CACHE["nc"] = _build_phase3_nc()
    return _CACHE["nc"]


def _pack128(a):
    """[S, 4, A] -> [128, 2, S] with p = anchor + 64*(b%2), s = b//2."""
    return np.ascontiguousarray(
        a.reshape(S_ENC, 2, 2, A).transpose(2, 3, 1, 0)       # [h, a, s, t]
    ).reshape(128, 2, S_ENC)


def _pack_inputs(Z, G_all):
    """Z [S,B,D] f32, G_all [S,B,A] f32 -> per-core in_maps."""
    import ml_dtypes
    bf16 = ml_dtypes.bfloat16
    f = np.float32
    Zq = np.clip(np.rint(Z * QS), -127, 127).astype(np.int8)  # [S,B,D]
    zsum = Zq.astype(f).sum(axis=2) / QS                      # [S,B]
    E = _make_E().astype(bf16)
    in_maps = []
    for i in range(N_CORES):
        bs = slice(4 * i, 4 * i + 4)
        zc = np.ascontiguousarray(Zq[:, bs, :].transpose(1, 0, 2))  # [BL,S,D]
        g = G_all[:, bs, :].astype(f)                         # [S,4,A]
        zs = zsum[:, bs]                                      # [S,4]
        m = g * (zs / D)[:, :, None]                          # m_t
        mp = np.broadcast_to((zs / D)[:, :, None], g.shape)   # m/g
        gsq = (g * g - 0.25) / (D - 1)
        v2 = D * m * m / (D - 1)
        gc = np.zeros_like(g)
        gc[:-1] = g[:-1] * (1.0 - g[1:]) / g[1:] - 0.5
        gfin = _pack128(np.broadcast_to(g[-1:], g.shape))[:, :, 0].astype(f)
        in_maps.append({
            "Z": zc, "E": E,
            "MP": _pack128(mp).astype(bf16),
            "GSQ": _pack128(gsq).astype(bf16),
            "V2": _pack128(v2).astype(bf16),
            "GC": _pack128(gc).astype(bf16),
            "GFIN": np.ascontiguousarray(gfin),
        })
    return in_maps


def _unpack_av(results):
    av = np.empty((B, A, D), np.float32)
    for i in range(N_CORES):
        o = np.asarray(results[i]["avout"], np.float32)   # [2, 128, D]
        for s in range(2):
            for h in range(2):
                av[4 * i + 2 * s + h] = o[s, 64 * h:64 * (h + 1), :]
    return av


def _phase3_on_trn(Z, G_all):
    from concourse.bass_utils import run_bass_kernel_spmd
    nc = _get_nc()
    in_maps = _pack_inputs(Z, G_all)
    for m in in_maps:
        for k in ("MP", "GSQ", "V2", "GC", "GFIN"):
            if not np.isfinite(np.asarray(m[k], np.float32)).all():
                raise ValueError("non-finite coefficient table")
    res = run_bass_kernel_spmd(nc, in_maps, core_ids=list(range(N_CORES)))
    av = _unpack_av(res.results)
    if not np.isfinite(av).all():
        raise ValueError("non-finite device output")
    return av


def _phase3_host(Z, G_all, n1_g, n1_b):
    """Fallback: vectorized numpy recurrence."""
    f = np.float32
    av = np.zeros((B, A, D), f)
    X = np.empty((B, A, D), f)
    for t in range(S_ENC):
        g = G_all[t][:, :, None]
        np.subtract(Z[t][:, None, :], av, out=X)
        X *= g
        av += X
        m = av.mean(-1, keepdims=True)
        av -= m
        q = np.einsum('bad,bad->ba', av, av)
        s = np.sqrt(q / (D - 1)) + EPS
        av /= s[:, :, None]
        if n1_g is not None:
            av *= n1_g
            av += n1_b
    return av


# --------------------------------------------------------------- model ----
def kernel(input_sequence, output_sequence, emb_in, emb_out, enc_key_W,
           enc_Wq, enc_bq, enc_Wk, enc_bk, n1_g, n1_b, dec_key_W,
           rdr_Wq, rdr_bq, rdr_Wk, rdr_bk, rdr_Wv, rdr_bv,
           dat_Wq, dat_bq, dat_Wk, dat_bk, n2_g, n2_b, n3_g, n3_b,
           voc_W, voc_b):
    f = np.float32
    emb_in = np.asarray(emb_in, f)
    scale = f(1.0 / math.sqrt(D))
    sqrtD = f(math.sqrt(D))
    idx = np.asarray(input_sequence)
    n1_g = np.asarray(n1_g, f)
    n1_b = np.asarray(n1_b, f)

    # -- encoder phase 1: z-trajectory (independent of av) --
    x_enc = emb_in[idx] * sqrtD                                # [B,S,D]
    Z = np.empty((S_ENC, B, D), f)
    z = np.zeros((B, D), f)
    for t in range(S_ENC):
        z = z + x_enc[:, t]
        m = z.mean(-1, keepdims=True)
        z -= m
        q = np.einsum('bd,bd->b', z, z)
        sd = np.sqrt(q / (D - 1)) + EPS
        z /= sd[:, None]
        if n1_g is not None:
            z *= n1_g
            z += n1_b
        Z[t] = z

    # -- encoder phase 2: batched gates (collapsed GEMM) --
    Qa = enc_key_W @ enc_Wq.T + enc_bq                         # [A,D]
    W2 = (enc_Wk.T @ Qa.T).astype(f)                           # [D,A]
    c2 = (enc_bk @ Qa.T).astype(f)                             # [A]
    G_all = _sigmoid((Z.reshape(-1, D) @ W2 + c2) * scale).reshape(
        S_ENC, B, A)

    # -- encoder phase 3: anchor-value recurrence on the NeuronCores --
    # device kernel computes plain LN; apply affine n1_g/n1_b after if
    # they are not identity (setup uses ones/zeros).
    affine = not (np.allclose(n1_g, 1.0) and np.allclose(n1_b, 0.0))
    if affine:
        av = _phase3_host(Z, G_all, n1_g, n1_b)
    else:
        try:
            av = _phase3_on_trn(Z, G_all)
        except Exception:
            av = _phase3_host(Z, G_all, None, None)

    # -- decoder (avx carry is dead code; z path only) --
    Kr = av @ rdr_Wk.T + rdr_bk                                # [B,A,D]
    Vr = av @ rdr_Wv.T + rdr_bv
    # fold the q-projection into the attention: s = zd @ M[b] + c[b]
    M = np.einsum('ed,bae->bda', np.asarray(rdr_Wq, f), Kr)    # [B,D,A]
    c = np.einsum('e,bae->ba', np.asarray(rdr_bq, f), Kr)      # [B,A]
    n2_g = np.asarray(n2_g, f)
    n2_b = np.asarray(n2_b, f)
    zd = Z[-1].copy()                                          # [B,D]
    for t in range(S_DEC):
        a = (np.einsum('bd,bda->ba', zd, M) + c) * scale       # [B,A]
        a -= a.max(axis=-1, keepdims=True)
        e = np.exp(a)
        e /= e.sum(axis=-1, keepdims=True)
        zd = zd + np.einsum('ba,bad->bd', e, Vr)
        m = zd.mean(-1, keepdims=True)
        zd -= m
        q = np.einsum('bd,bd->b', zd, zd)
        sd = np.sqrt(q / (D - 1)) + EPS
        zd /= sd[:, None]
        zd *= n2_g
        zd += n2_b

    # -- logits + log_softmax on host --
    zfin = zd.astype(f)                                        # [B,D]
    logits = zfin @ np.asarray(voc_W, f).T + voc_b             # [B,V]
    logits = logits[:, None, :]
    mx = logits.max(axis=-1, keepdims=True)
    lse = np.log(np.exp(logits - mx).sum(axis=-1, keepdims=True)) + mx
    return (logits - lse).astype(f)


# ------------------------------------------------------------- profile ----
def _profile():
    """Best-available timing of the bass kernel: HW NTFF if possible,
    else CoreSim cost-model time. Returns (exec_ns, source)."""
    nc = _get_nc()
    rng = np.random.default_rng(0)
    Z = rng.standard_normal((S_ENC, B, D)).astype(np.float32)
    G = (0.5 + 0.01 * rng.standard_normal((S_ENC, B, A))).astype(np.float32)
    in_maps = _pack_inputs(Z, G)
    try:
        from concourse.bass_utils import run_bass_kernel_spmd
        res = run_bass_kernel_spmd(nc, in_maps,
                                   core_ids=list(range(N_CORES)), trace=True)
        if res.exec_time_ns:
            return res.exec_time_ns, "hw-ntff"
    except Exception:
        pass
    from concourse.bass_interp import CoreSim
    sim = CoreSim(nc, publish_trace=False)
    for k, v in in_maps[0].items():
        sim.tensor(k)[:] = v
    sim.simulate()
    return int(sim.time), "coresim"


# revision 31
# speedup vs baseline: 1.0861x; 1.0063x over previous
"""AnchorOnlyMixtureRNN — 8-core Trainium2 kernel.

Architecture (scatter_memory): the model is two sequential scans plus dense
ops. The dominant cost — the 1024-step anchor-value (av) gated-LayerNorm
recurrence over state [B, A=64, D=512] — runs on the 8 NeuronCores, batch-
sharded 4 batches/core (pure data parallelism: the scan state is per-batch).
The cheap/BLAS-friendly parts (embedding gather, z-trajectory, collapsed
gate GEMM, 256-step decoder, vocab logits, log-softmax) run on host.

Device kernel per core (B_local=4):
  chains (b, a) -> tile s = b//2, partition p = a + 64*(b%2)
  - Z ships int8 (scale 1/16; dequant folded into the E selection matmul),
    staged from DRAM in 64-step blocks via SWDGE cast-DMA to [4, 64, 512].
  - per step: zb_s = E_s.T @ Zstage[:, j, :] on TensorE -> PSUM;
    state kept as u = x/g so the blend is u = (u_prev - MP)*C' + zb
    (one tensor_scalar + one tensor_tensor on VectorE per tile);
    sum(x) is analytic (host-shipped tables), sumsq via ScalarE Square
    with accum; rstd = 1/std via two Newton iterations seeded from the
    previous step (exact ACT sqrt for the first 8 transient steps, h
    clamped on the first iteration to survive variance jumps).
  - per-step coefficient tables (MP, GSQ, V2, GC) are host-precomputed,
    delta-encoded around their constant centers so bf16 keeps the tiny
    anchor-differentiating gate signal, and cast to f32 on device once.
  - final normalize + bf16 DMA out.

Walrus in this toolchain allows at most ONE sync wait per instruction:
the kernel must be built with bacc.Bacc (its finalize pipeline splits
waits via event semaphores); plain bass.Bass modules fail codegen.
tensor_tensor_reduce crashes the device at runtime — avoided.
"""
import math
import numpy as np

D = 512
A = 64
V_OUT = 32000
B = 32
S_ENC = 1024
S_DEC = 256
EPS = 1e-6
N_CORES = 8
BL = B // N_CORES          # 4 local batches per core
TBLK = 16                  # z staging block (steps; small so each
                           # Pool cast-DMA burst fits the per-step slack)
QS = 16.0                  # int8 quant scale for Z
KG = 8000.0                # int8 quant scale for (g - 0.5)

_CACHE = {}


def _ln(x, g, b):
    m = x.mean(axis=-1, keepdims=True)
    s = x.std(axis=-1, ddof=1, keepdims=True)
    return g * (x - m) / (s + EPS) + b


def _sigmoid(x):
    return 1.0 / (1.0 + np.exp(-x))


# ---------------------------------------------------------------- Bass ----
def _build_phase3_nc(S=S_ENC):
    import concourse.bacc as bacc
    import concourse.tile as tile
    from concourse import mybir

    f32 = mybir.dt.float32
    bf16 = mybir.dt.bfloat16
    i8 = mybir.dt.int8
    Alu = mybir.AluOpType
    Act = mybir.ActivationFunctionType

    from concourse.tile_rust import add_dep_helper

    nc = bacc.Bacc("TRN2", target_bir_lowering=False)
    Z_d = nc.declare_dram_parameter("Z", [BL, S, D], i8, isOutput=False)
    E_d = nc.declare_dram_parameter("E", [BL, 2, 128], bf16, isOutput=False)
    # per-step coefficient tables (host-precomputed, see _pack_inputs):
    #   MP[.,.,t]  = zsum_t/512            (= m_t/g_t)
    #   GSQ[.,.,t] = g_t^2
    #   V2[.,.,t]  = 512*m_t^2
    #   GC[.,.,t]  = g_t*(1-g_{t+1})/g_{t+1}
    MP_d = nc.declare_dram_parameter("MP", [128, 2, S], bf16, isOutput=False)
    GSQ_d = nc.declare_dram_parameter("GSQ", [128, 2, S], bf16, isOutput=False)
    V2_d = nc.declare_dram_parameter("V2", [128, 2, S], bf16, isOutput=False)
    GC_d = nc.declare_dram_parameter("GC", [128, 2, S], bf16, isOutput=False)
    GFIN_d = nc.declare_dram_parameter("GFIN", [128, 2], f32, isOutput=False)
    out_d = nc.declare_dram_parameter("avout", [2, 128, D], bf16, isOutput=True)

    with tile.TileContext(nc) as tc:
        with (
            tc.tile_pool(name="big", bufs=1) as big,
            tc.tile_pool(name="stage", bufs=3) as stg,
            tc.tile_pool(name="work", bufs=2) as work,
            tc.tile_pool(name="st", bufs=2) as stp,
            tc.tile_pool(name="ps", bufs=4, space="PSUM") as ps,
        ):
            Et = big.tile([BL, 2, 128], bf16, tag="E")
            nc.sync.dma_start(Et[:], E_d[:])
            GFINt = big.tile([128, 2], f32, tag="GFIN")
            nc.sync.dma_start(GFINt[:], GFIN_d[:])
            # load coefficient tables, cast bf16 -> f32 (scalar-AP operands)
            tabs = {}
            centers = {"MP": None, "GSQ": 0.25 / 511.0, "V2": None,
                       "GC": 0.5}
            for nm, dram in (("MP", MP_d), ("GSQ", GSQ_d), ("V2", V2_d),
                             ("GC", GC_d)):
                tb = big.tile([128, 2, S], bf16, name=f"{nm}b", tag=f"{nm}b")
                nc.sync.dma_start(tb[:], dram[:])
                tf = big.tile([128, 2, S], f32, name=f"{nm}f", tag=f"{nm}f")
                if centers[nm] is None:
                    nc.vector.tensor_copy(tf[:], tb[:])
                else:
                    nc.vector.tensor_scalar(tf[:], tb[:], centers[nm], None,
                                            Alu.add)
                tabs[nm] = tf
            MPt, GSQt, V2t, GCt = (tabs[k] for k in ("MP", "GSQ", "V2", "GC"))

            # persistent state: u = x/g per tile, double-buffered so the
            # scalar-engine Square read of step t doesn't block step t+1's
            # state write (WAR)
            u = [[big.tile([128, D], f32, name=f"us{s}{k}", tag=f"us{s}{k}")
                  for k in range(2)] for s in range(2)]
            Cp = [None, None]      # per-tile blend coefficient
            rstd_p = [None, None]  # per-tile rstd (Newton seed)

            def emit_stats(t, s, s2, after=None):
                # stats chain on the (otherwise idle) GpSimd engine so its
                # ACT-wait never stalls the in-order DVE stream
                var = stp.tile([128, 1], f32, name=f"var{s}", tag=f"var{s}")
                nc.gpsimd.tensor_scalar(
                    var[:], s2[:], GSQt[:, s, t:t + 1],
                    V2t[:, s, t:t + 1], Alu.mult, Alu.subtract)
                rstd = stp.tile([128, 1], f32, name=f"rstd{s}",
                                tag=f"rstd{s}")
                if t < 8:
                    std = stp.tile([128, 1], f32, name=f"std{s}",
                                   tag=f"std{s}")
                    nc.scalar.activation(std[:], var[:], Act.Sqrt)
                    stde = stp.tile([128, 1], f32, name=f"stde{s}",
                                    tag=f"stde{s}")
                    nc.vector.tensor_scalar(stde[:], std[:], EPS, None,
                                            Alu.add)
                    nc.vector.reciprocal(rstd[:], stde[:])
                else:
                    # one clamped Newton rsqrt iteration from prev rstd
                    r = rstd_p[s]
                    r2 = stp.tile([128, 1], f32, name=f"nr2{s}",
                                  tag=f"nr2{s}")
                    nc.gpsimd.tensor_tensor(r2[:], r[:], r[:], Alu.mult)
                    w = stp.tile([128, 1], f32, name=f"nw{s}", tag=f"nw{s}")
                    nc.gpsimd.tensor_tensor(w[:], var[:], r2[:], Alu.mult)
                    h = stp.tile([128, 1], f32, name=f"nh{s}", tag=f"nh{s}")
                    nc.gpsimd.tensor_scalar(h[:], w[:], -0.5, 1.5,
                                            Alu.mult, Alu.add)
                    nc.gpsimd.tensor_scalar(h[:], h[:], 0.25, None, Alu.max)
                    nc.gpsimd.tensor_tensor(rstd[:], r[:], h[:], Alu.mult)
                Cn = stp.tile([128, 1], f32, name=f"Cn{s}", tag=f"Cn{s}")
                if t + 1 < S:
                    nc.gpsimd.tensor_tensor(Cn[:], GCt[:, s, t:t + 1],
                                            rstd[:], Alu.mult)
                else:
                    nc.gpsimd.tensor_tensor(Cn[:], GFINt[:, s:s + 1],
                                            rstd[:], Alu.mult)
                Cp[s] = Cn
                rstd_p[s] = rstd

            pending = None
            nblk = (S + TBLK - 1) // TBLK
            QRT = TBLK // 4
            zst_tiles = {}

            def emit_stage(blk, q):
                # quarter-slice staging for block `blk` (cast int8->bf16);
                # spread across earlier steps so each Pool burst fits slack
                if blk >= nblk:
                    return
                if q == 0:
                    zst_tiles[blk] = stg.tile([BL, TBLK, D], bf16, name="zst", tag="zst")
                t0b = blk * TBLK
                lo = q * QRT
                hi = min((q + 1) * QRT, S - t0b)
                if lo < hi:
                    nc.gpsimd.dma_start(
                        zst_tiles[blk][:, lo:hi, :],
                        Z_d[:, t0b + lo:t0b + hi, :])

            for blk in range(2):
                for q in range(4):
                    emit_stage(blk, q)
            for blk in range(nblk):
                t0b = blk * TBLK
                nstep = min(TBLK, S - t0b)
                zst = zst_tiles.pop(blk)
                for j in range(nstep):
                    t = t0b + j
                    for s in range(2):
                        zbs = ps.tile([128, D], f32, name=f"zb{s}",
                                      tag=f"zb{s}")
                        nc.tensor.matmul(zbs[:], Et[:, s, :], zst[:, j, :],
                                         start=True, stop=True)
                        ucur = u[s][t % 2]
                        if t == 0:
                            nc.vector.tensor_copy(ucur[:], zbs[:])
                        else:
                            # y' = (u_prev - MP_{t-1}) * C' ; u = y' + zb
                            y = work.tile([128, D], f32, name=f"y{s}",
                                          tag=f"y{s}")
                            nc.vector.tensor_scalar(
                                y[:], u[s][(t - 1) % 2][:],
                                MPt[:, s, t - 1:t],
                                Cp[s][:], Alu.subtract, Alu.mult)
                            nc.vector.tensor_tensor(ucur[:], y[:], zbs[:],
                                                    Alu.add)
                        s2 = stp.tile([128, 1], f32, name=f"s2{s}",
                                      tag=f"s2{s}")
                        usq = work.tile([128, D], f32, name=f"usq{s}",
                                        tag=f"usq{s}")
                        nc.scalar.activation(usq[:], ucur[:], Act.Square,
                                             accum_out=s2[:])
                        if pending is not None:
                            emit_stats(*pending)
                        pending = (t, s, s2)
                    # prefetch block blk+2, one quarter per two steps
                    if j in (1, 3, 5, 7):
                        emit_stage(blk + 2, (j - 1) // 2)
            emit_stats(*pending)

            # final normalize: av = (u - MP_{S-1}) * (g_{S-1}*rstd), DMA out
            for s in range(2):
                avf = work.tile([128, D], bf16, name=f"avf{s}", tag=f"avf{s}")
                nc.vector.tensor_scalar(
                    avf[:], u[s][(S - 1) % 2][:], MPt[:, s, S - 1:S],
                    Cp[s][:], Alu.subtract, Alu.mult)
                nc.sync.dma_start(out_d[s], avf[:])
    nc.finalize()
    return nc


def _make_E():
    E = np.zeros((BL, 2, 128), np.float32)
    for s in range(2):
        for h in range(2):
            E[2 * s + h, s, 64 * h:64 * (h + 1)] = 1.0 / QS
    return E


def _get_nc():
    if "nc" not in _CACHE:
        _CACHE["nc"] = _build_phase3_nc()
    return _CACHE["nc"]


def _pack128(a):
    """[S, 4, A] -> [128, 2, S] with p = anchor + 64*(b%2), s = b//2."""
    return np.ascontiguousarray(
        a.reshape(S_ENC, 2, 2, A).transpose(2, 3, 1, 0)       # [h, a, s, t]
    ).reshape(128, 2, S_ENC)


def _pack_inputs(Z, G_all):
    """Z [S,B,D] f32, G_all [S,B,A] f32 -> per-core in_maps."""
    import ml_dtypes
    bf16 = ml_dtypes.bfloat16
    f = np.float32
    Zq = np.clip(np.rint(Z * QS), -127, 127).astype(np.int8)  # [S,B,D]
    zsum = Zq.astype(f).sum(axis=2) / QS                      # [S,B]
    E = _make_E().astype(bf16)
    in_maps = []
    for i in range(N_CORES):
        bs = slice(4 * i, 4 * i + 4)
        zc = np.ascontiguousarray(Zq[:, bs, :].transpose(1, 0, 2))  # [BL,S,D]
        g = G_all[:, bs, :].astype(f)                         # [S,4,A]
        zs = zsum[:, bs]                                      # [S,4]
        m = g * (zs / D)[:, :, None]                          # m_t
        mp = np.broadcast_to((zs / D)[:, :, None], g.shape)   # m/g
        gsq = (g * g - 0.25) / (D - 1)
        v2 = D * m * m / (D - 1)
        gc = np.zeros_like(g)
        gc[:-1] = g[:-1] * (1.0 - g[1:]) / g[1:] - 0.5
        gfin = _pack128(np.broadcast_to(g[-1:], g.shape))[:, :, 0].astype(f)
        in_maps.append({
            "Z": zc, "E": E,
            "MP": _pack128(mp).astype(bf16),
            "GSQ": _pack128(gsq).astype(bf16),
            "V2": _pack128(v2).astype(bf16),
            "GC": _pack128(gc).astype(bf16),
            "GFIN": np.ascontiguousarray(gfin),
        })
    return in_maps


def _unpack_av(results):
    av = np.empty((B, A, D), np.float32)
    for i in range(N_CORES):
        o = np.asarray(results[i]["avout"], np.float32)   # [2, 128, D]
        for s in range(2):
            for h in range(2):
                av[4 * i + 2 * s + h] = o[s, 64 * h:64 * (h + 1), :]
    return av


def _phase3_on_trn(Z, G_all):
    from concourse.bass_utils import run_bass_kernel_spmd
    nc = _get_nc()
    in_maps = _pack_inputs(Z, G_all)
    for m in in_maps:
        for k in ("MP", "GSQ", "V2", "GC", "GFIN"):
            if not np.isfinite(np.asarray(m[k], np.float32)).all():
                raise ValueError("non-finite coefficient table")
    res = run_bass_kernel_spmd(nc, in_maps, core_ids=list(range(N_CORES)))
    av = _unpack_av(res.results)
    if not np.isfinite(av).all():
        raise ValueError("non-finite device output")
    return av


def _phase3_host(Z, G_all, n1_g, n1_b):
    """Fallback: vectorized numpy recurrence."""
    f = np.float32
    av = np.zeros((B, A, D), f)
    X = np.empty((B, A, D), f)
    for t in range(S_ENC):
        g = G_all[t][:, :, None]
        np.subtract(Z[t][:, None, :], av, out=X)
        X *= g
        av += X
        m = av.mean(-1, keepdims=True)
        av -= m
        q = np.einsum('bad,bad->ba', av, av)
        s = np.sqrt(q / (D - 1)) + EPS
        av /= s[:, :, None]
        if n1_g is not None:
            av *= n1_g
            av += n1_b
    return av


# --------------------------------------------------------------- model ----
def kernel(input_sequence, output_sequence, emb_in, emb_out, enc_key_W,
           enc_Wq, enc_bq, enc_Wk, enc_bk, n1_g, n1_b, dec_key_W,
           rdr_Wq, rdr_bq, rdr_Wk, rdr_bk, rdr_Wv, rdr_bv,
           dat_Wq, dat_bq, dat_Wk, dat_bk, n2_g, n2_b, n3_g, n3_b,
           voc_W, voc_b):
    f = np.float32
    emb_in = np.asarray(emb_in, f)
    scale = f(1.0 / math.sqrt(D))
    sqrtD = f(math.sqrt(D))
    idx = np.asarray(input_sequence)
    n1_g = np.asarray(n1_g, f)
    n1_b = np.asarray(n1_b, f)

    # -- encoder phase 1: z-trajectory (independent of av) --
    x_enc = emb_in[idx] * sqrtD                                # [B,S,D]
    Z = np.empty((S_ENC, B, D), f)
    z = np.zeros((B, D), f)
    for t in range(S_ENC):
        z = z + x_enc[:, t]
        m = z.mean(-1, keepdims=True)
        z -= m
        q = np.einsum('bd,bd->b', z, z)
        sd = np.sqrt(q / (D - 1)) + EPS
        z /= sd[:, None]
        if n1_g is not None:
            z *= n1_g
            z += n1_b
        Z[t] = z

    # -- encoder phase 2: batched gates (collapsed GEMM) --
    Qa = enc_key_W @ enc_Wq.T + enc_bq                         # [A,D]
    W2 = (enc_Wk.T @ Qa.T).astype(f)                           # [D,A]
    c2 = (enc_bk @ Qa.T).astype(f)                             # [A]
    G_all = _sigmoid((Z.reshape(-1, D) @ W2 + c2) * scale).reshape(
        S_ENC, B, A)

    # -- encoder phase 3: anchor-value recurrence on the NeuronCores --
    # device kernel computes plain LN; apply affine n1_g/n1_b after if
    # they are not identity (setup uses ones/zeros).
    affine = not (np.allclose(n1_g, 1.0) and np.allclose(n1_b, 0.0))
    if affine:
        av = _phase3_host(Z, G_all, n1_g, n1_b)
    else:
        try:
            av = _phase3_on_trn(Z, G_all)
        except Exception:
            av = _phase3_host(Z, G_all, None, None)

    # -- decoder (avx carry is dead code; z path only) --
    Kr = av @ rdr_Wk.T + rdr_bk                                # [B,A,D]
    Vr = av @ rdr_Wv.T + rdr_bv
    # fold the q-projection into the attention: s = zd @ M[b] + c[b]
    M = np.einsum('ed,bae->bda', np.asarray(rdr_Wq, f), Kr)    # [B,D,A]
    c = np.einsum('e,bae->ba', np.asarray(rdr_bq, f), Kr)      # [B,A]
    n2_g = np.asarray(n2_g, f)
    n2_b = np.asarray(n2_b, f)
    zd = Z[-1].copy()                                          # [B,D]
    for t in range(S_DEC):
        a = (np.einsum('bd,bda->ba', zd, M) + c) * scale       # [B,A]
        a -= a.max(axis=-1, keepdims=True)
        e = np.exp(a)
        e /= e.sum(axis=-1, keepdims=True)
        zd = zd + np.einsum('ba,bad->bd', e, Vr)
        m = zd.mean(-1, keepdims=True)
        zd -= m
        q = np.einsum('bd,bd->b', zd, zd)
        sd = np.sqrt(q / (D - 1)) + EPS
        zd /= sd[:, None]
        zd *= n2_g
        zd += n2_b

    # -- logits + log_softmax on host --
    zfin = zd.astype(f)                                        # [B,D]
    logits = zfin @ np.asarray(voc_W, f).T + voc_b             # [B,V]
    logits = logits[:, None, :]
    mx = logits.max(axis=-1, keepdims=True)
    lse = np.log(np.exp(logits - mx).sum(axis=-1, keepdims=True)) + mx
    return (logits - lse).astype(f)


# ------------------------------------------------------------- profile ----
def _profile():
    """Best-available timing of the bass kernel: HW NTFF if possible,
    else CoreSim cost-model time. Returns (exec_ns, source)."""
    nc = _get_nc()
    rng = np.random.default_rng(0)
    Z = rng.standard_normal((S_ENC, B, D)).astype(np.float32)
    G = (0.5 + 0.01 * rng.standard_normal((S_ENC, B, A))).astype(np.float32)
    in_maps = _pack_inputs(Z, G)
    try:
        from concourse.bass_utils import run_bass_kernel_spmd
        res = run_bass_kernel_spmd(nc, in_maps,
                                   core_ids=list(range(N_CORES)), trace=True)
        if res.exec_time_ns:
            return res.exec_time_ns, "hw-ntff"
    except Exception:
        pass
    from concourse.bass_interp import CoreSim
    sim = CoreSim(nc, publish_trace=False)
    for k, v in in_maps[0].items():
        sim.tensor(k)[:] = v
    sim.simulate()
    return int(sim.time), "coresim"


# revision 33
# speedup vs baseline: 1.1084x; 1.0205x over previous
"""AnchorOnlyMixtureRNN — 8-core Trainium2 kernel.

Architecture (scatter_memory): the model is two sequential scans plus dense
ops. The dominant cost — the 1024-step anchor-value (av) gated-LayerNorm
recurrence over state [B, A=64, D=512] — runs on the 8 NeuronCores, batch-
sharded 4 batches/core (pure data parallelism: the scan state is per-batch).
The cheap/BLAS-friendly parts (embedding gather, z-trajectory, collapsed
gate GEMM, 256-step decoder, vocab logits, log-softmax) run on host.

Device kernel per core (B_local=4):
  chains (b, a) -> tile s = b//2, partition p = a + 64*(b%2)
  - Z ships int8 (scale 1/16; dequant folded into the E selection matmul),
    staged from DRAM in 64-step blocks via SWDGE cast-DMA to [4, 64, 512].
  - per step: zb_s = E_s.T @ Zstage[:, j, :] on TensorE -> PSUM;
    state kept as u = x/g so the blend is u = (u_prev - MP)*C' + zb
    (one tensor_scalar + one tensor_tensor on VectorE per tile);
    sum(x) is analytic (host-shipped tables), sumsq via ScalarE Square
    with accum; rstd = 1/std via two Newton iterations seeded from the
    previous step (exact ACT sqrt for the first 8 transient steps, h
    clamped on the first iteration to survive variance jumps).
  - per-step coefficient tables (MP, GSQ, V2, GC) are host-precomputed,
    delta-encoded around their constant centers so bf16 keeps the tiny
    anchor-differentiating gate signal, and cast to f32 on device once.
  - final normalize + bf16 DMA out.

Walrus in this toolchain allows at most ONE sync wait per instruction:
the kernel must be built with bacc.Bacc (its finalize pipeline splits
waits via event semaphores); plain bass.Bass modules fail codegen.
tensor_tensor_reduce crashes the device at runtime — avoided.
"""
import math
import numpy as np

D = 512
A = 64
V_OUT = 32000
B = 32
S_ENC = 1024
S_DEC = 256
EPS = 1e-6
N_CORES = 8
BL = B // N_CORES          # 4 local batches per core
TBLK = 16                  # z staging block (steps; small so each
                           # Pool cast-DMA burst fits the per-step slack)
QS = 16.0                  # int8 quant scale for Z
KG = 8000.0                # int8 quant scale for (g - 0.5)

_CACHE = {}


def _ln(x, g, b):
    m = x.mean(axis=-1, keepdims=True)
    s = x.std(axis=-1, ddof=1, keepdims=True)
    return g * (x - m) / (s + EPS) + b


def _sigmoid(x):
    return 1.0 / (1.0 + np.exp(-x))


# ---------------------------------------------------------------- Bass ----
def _build_phase3_nc(S=S_ENC):
    import concourse.bacc as bacc
    import concourse.tile as tile
    from concourse import mybir

    f32 = mybir.dt.float32
    bf16 = mybir.dt.bfloat16
    i8 = mybir.dt.int8
    Alu = mybir.AluOpType
    Act = mybir.ActivationFunctionType

    from concourse.tile_rust import add_dep_helper

    nc = bacc.Bacc("TRN2", target_bir_lowering=False)
    Z_d = nc.declare_dram_parameter("Z", [BL, S, D], i8, isOutput=False)
    E_d = nc.declare_dram_parameter("E", [BL, 2, 128], bf16, isOutput=False)
    # per-step coefficient tables (host-precomputed, see _pack_inputs):
    #   MP[.,.,t]  = zsum_t/512            (= m_t/g_t)
    #   GSQ[.,.,t] = g_t^2
    #   V2[.,.,t]  = 512*m_t^2
    #   GC[.,.,t]  = g_t*(1-g_{t+1})/g_{t+1}
    MP_d = nc.declare_dram_parameter("MP", [128, 2, S], bf16, isOutput=False)
    GSQ_d = nc.declare_dram_parameter("GSQ", [128, 2, S], bf16, isOutput=False)
    V2_d = nc.declare_dram_parameter("V2", [128, 2, S], bf16, isOutput=False)
    GC_d = nc.declare_dram_parameter("GC", [128, 2, S], bf16, isOutput=False)
    GFIN_d = nc.declare_dram_parameter("GFIN", [128, 2], f32, isOutput=False)
    out_d = nc.declare_dram_parameter("avout", [2, 128, D], bf16, isOutput=True)

    with tile.TileContext(nc) as tc:
        with (
            tc.tile_pool(name="big", bufs=1) as big,
            tc.tile_pool(name="stage", bufs=1) as stg,
            tc.tile_pool(name="work", bufs=2) as work,
            tc.tile_pool(name="st", bufs=2) as stp,
            tc.tile_pool(name="ps", bufs=4, space="PSUM") as ps,
        ):
            Et = big.tile([BL, 2, 128], bf16, tag="E")
            nc.sync.dma_start(Et[:], E_d[:])
            GFINt = big.tile([128, 2], f32, tag="GFIN")
            nc.sync.dma_start(GFINt[:], GFIN_d[:])
            # load coefficient tables, cast bf16 -> f32 (scalar-AP operands)
            tabs = {}
            centers = {"MP": None, "GSQ": 0.25 / 511.0, "V2": None,
                       "GC": 0.5}
            for nm, dram in (("MP", MP_d), ("GSQ", GSQ_d), ("V2", V2_d),
                             ("GC", GC_d)):
                tb = big.tile([128, 2, S], bf16, name=f"{nm}b", tag=f"{nm}b")
                nc.sync.dma_start(tb[:], dram[:])
                tf = big.tile([128, 2, S], f32, name=f"{nm}f", tag=f"{nm}f")
                if centers[nm] is None:
                    nc.vector.tensor_copy(tf[:], tb[:])
                else:
                    nc.vector.tensor_scalar(tf[:], tb[:], centers[nm], None,
                                            Alu.add)
                tabs[nm] = tf
            MPt, GSQt, V2t, GCt = (tabs[k] for k in ("MP", "GSQ", "V2", "GC"))

            # persistent state: u = x/g per tile, double-buffered so the
            # scalar-engine Square read of step t doesn't block step t+1's
            # state write (WAR)
            u = [[big.tile([128, D], f32, name=f"us{s}{k}", tag=f"us{s}{k}")
                  for k in range(2)] for s in range(2)]
            Cp = [None, None]      # per-tile blend coefficient
            rstd_p = [None, None]  # per-tile rstd (Newton seed)

            def emit_stats(t, s, s2, after=None):
                # stats chain on the (otherwise idle) GpSimd engine so its
                # ACT-wait never stalls the in-order DVE stream
                var = stp.tile([128, 1], f32, name=f"var{s}", tag=f"var{s}")
                nc.gpsimd.tensor_scalar(
                    var[:], s2[:], GSQt[:, s, t:t + 1],
                    V2t[:, s, t:t + 1], Alu.mult, Alu.subtract)
                rstd = stp.tile([128, 1], f32, name=f"rstd{s}",
                                tag=f"rstd{s}")
                if t < 8:
                    std = stp.tile([128, 1], f32, name=f"std{s}",
                                   tag=f"std{s}")
                    nc.scalar.activation(std[:], var[:], Act.Sqrt)
                    stde = stp.tile([128, 1], f32, name=f"stde{s}",
                                    tag=f"stde{s}")
                    nc.vector.tensor_scalar(stde[:], std[:], EPS, None,
                                            Alu.add)
                    nc.vector.reciprocal(rstd[:], stde[:])
                else:
                    # one clamped Newton rsqrt iteration from prev rstd
                    r = rstd_p[s]
                    r2 = stp.tile([128, 1], f32, name=f"nr2{s}",
                                  tag=f"nr2{s}")
                    nc.gpsimd.tensor_tensor(r2[:], r[:], r[:], Alu.mult)
                    w = stp.tile([128, 1], f32, name=f"nw{s}", tag=f"nw{s}")
                    nc.gpsimd.tensor_tensor(w[:], var[:], r2[:], Alu.mult)
                    h = stp.tile([128, 1], f32, name=f"nh{s}", tag=f"nh{s}")
                    nc.gpsimd.tensor_scalar(h[:], w[:], -0.5, 1.5,
                                            Alu.mult, Alu.add)
                    nc.gpsimd.tensor_scalar(h[:], h[:], 0.25, None, Alu.max)
                    nc.gpsimd.tensor_tensor(rstd[:], r[:], h[:], Alu.mult)
                Cn = stp.tile([128, 1], f32, name=f"Cn{s}", tag=f"Cn{s}")
                if t + 1 < S:
                    nc.gpsimd.tensor_tensor(Cn[:], GCt[:, s, t:t + 1],
                                            rstd[:], Alu.mult)
                else:
                    nc.gpsimd.tensor_tensor(Cn[:], GFINt[:, s:s + 1],
                                            rstd[:], Alu.mult)
                Cp[s] = Cn
                rstd_p[s] = rstd

            pending = None
            nblk = (S + TBLK - 1) // TBLK
            QRT = TBLK // 4
            # fixed ring of staging tiles (no per-block pool alloc/release)
            zring = [big.tile([BL, TBLK, D], bf16, name=f"zr{k}",
                              tag=f"zr{k}") for k in range(4)]

            def emit_stage(blk, q):
                # quarter-slice staging for block `blk` (cast int8->bf16)
                if blk >= nblk:
                    return
                t0b = blk * TBLK
                lo = q * QRT
                hi = min((q + 1) * QRT, S - t0b)
                if lo < hi:
                    nc.gpsimd.dma_start(
                        zring[blk % 4][:, lo:hi, :],
                        Z_d[:, t0b + lo:t0b + hi, :])

            for blk in range(2):
                for q in range(4):
                    emit_stage(blk, q)
            for blk in range(nblk):
                t0b = blk * TBLK
                nstep = min(TBLK, S - t0b)
                zst = zring[blk % 4]
                for j in range(nstep):
                    t = t0b + j
                    for s in range(2):
                        zbs = ps.tile([128, D], f32, name=f"zb{s}",
                                      tag=f"zb{s}")
                        nc.tensor.matmul(zbs[:], Et[:, s, :], zst[:, j, :],
                                         start=True, stop=True)
                        ucur = u[s][t % 2]
                        if t == 0:
                            nc.vector.tensor_copy(ucur[:], zbs[:])
                        else:
                            # y' = (u_prev - MP_{t-1}) * C' ; u = y' + zb
                            y = work.tile([128, D], f32, name=f"y{s}",
                                          tag=f"y{s}")
                            nc.vector.tensor_scalar(
                                y[:], u[s][(t - 1) % 2][:],
                                MPt[:, s, t - 1:t],
                                Cp[s][:], Alu.subtract, Alu.mult)
                            nc.vector.tensor_tensor(ucur[:], y[:], zbs[:],
                                                    Alu.add)
                        s2 = stp.tile([128, 1], f32, name=f"s2{s}",
                                      tag=f"s2{s}")
                        usq = work.tile([128, D], f32, name=f"usq{s}",
                                        tag=f"usq{s}")
                        nc.scalar.activation(usq[:], ucur[:], Act.Square,
                                             accum_out=s2[:])
                        if pending is not None:
                            emit_stats(*pending)
                        pending = (t, s, s2)
                    # prefetch block blk+2, one quarter per two steps
                    if j in (1, 3, 5, 7):
                        emit_stage(blk + 2, (j - 1) // 2)
            emit_stats(*pending)

            # final normalize: av = (u - MP_{S-1}) * (g_{S-1}*rstd), DMA out
            for s in range(2):
                avf = work.tile([128, D], bf16, name=f"avf{s}", tag=f"avf{s}")
                nc.vector.tensor_scalar(
                    avf[:], u[s][(S - 1) % 2][:], MPt[:, s, S - 1:S],
                    Cp[s][:], Alu.subtract, Alu.mult)
                nc.sync.dma_start(out_d[s], avf[:])
    nc.finalize()
    return nc


def _make_E():
    E = np.zeros((BL, 2, 128), np.float32)
    for s in range(2):
        for h in range(2):
            E[2 * s + h, s, 64 * h:64 * (h + 1)] = 1.0 / QS
    return E


def _get_nc():
    if "nc" not in _CACHE:
        _CACHE["nc"] = _build_phase3_nc()
    return _CACHE["nc"]


def _pack128(a):
    """[S, 4, A] -> [128, 2, S] with p = anchor + 64*(b%2), s = b//2."""
    return np.ascontiguousarray(
        a.reshape(S_ENC, 2, 2, A).transpose(2, 3, 1, 0)       # [h, a, s, t]
    ).reshape(128, 2, S_ENC)


def _pack_inputs(Z, G_all):
    """Z [S,B,D] f32, G_all [S,B,A] f32 -> per-core in_maps."""
    import ml_dtypes
    bf16 = ml_dtypes.bfloat16
    f = np.float32
    Zq = np.clip(np.rint(Z * QS), -127, 127).astype(np.int8)  # [S,B,D]
    zsum = Zq.astype(f).sum(axis=2) / QS                      # [S,B]
    E = _make_E().astype(bf16)
    in_maps = []
    for i in range(N_CORES):
        bs = slice(4 * i, 4 * i + 4)
        zc = np.ascontiguousarray(Zq[:, bs, :].transpose(1, 0, 2))  # [BL,S,D]
        g = G_all[:, bs, :].astype(f)                         # [S,4,A]
        zs = zsum[:, bs]                                      # [S,4]
        m = g * (zs / D)[:, :, None]                          # m_t
        mp = np.broadcast_to((zs / D)[:, :, None], g.shape)   # m/g
        gsq = (g * g - 0.25) / (D - 1)
        v2 = D * m * m / (D - 1)
        gc = np.zeros_like(g)
        gc[:-1] = g[:-1] * (1.0 - g[1:]) / g[1:] - 0.5
        gfin = _pack128(np.broadcast_to(g[-1:], g.shape))[:, :, 0].astype(f)
        in_maps.append({
            "Z": zc, "E": E,
            "MP": _pack128(mp).astype(bf16),
            "GSQ": _pack128(gsq).astype(bf16),
            "V2": _pack128(v2).astype(bf16),
            "GC": _pack128(gc).astype(bf16),
            "GFIN": np.ascontiguousarray(gfin),
        })
    return in_maps


def _unpack_av(results):
    av = np.empty((B, A, D), np.float32)
    for i in range(N_CORES):
        o = np.asarray(results[i]["avout"], np.float32)   # [2, 128, D]
        for s in range(2):
            for h in range(2):
                av[4 * i + 2 * s + h] = o[s, 64 * h:64 * (h + 1), :]
    return av


def _phase3_on_trn(Z, G_all):
    from concourse.bass_utils import run_bass_kernel_spmd
    nc = _get_nc()
    in_maps = _pack_inputs(Z, G_all)
    for m in in_maps:
        for k in ("MP", "GSQ", "V2", "GC", "GFIN"):
            if not np.isfinite(np.asarray(m[k], np.float32)).all():
                raise ValueError("non-finite coefficient table")
    res = run_bass_kernel_spmd(nc, in_maps, core_ids=list(range(N_CORES)))
    av = _unpack_av(res.results)
    if not np.isfinite(av).all():
        raise ValueError("non-finite device output")
    return av


def _phase3_host(Z, G_all, n1_g, n1_b):
    """Fallback: vectorized numpy recurrence."""
    f = np.float32
    av = np.zeros((B, A, D), f)
    X = np.empty((B, A, D), f)
    for t in range(S_ENC):
        g = G_all[t][:, :, None]
        np.subtract(Z[t][:, None, :], av, out=X)
        X *= g
        av += X
        m = av.mean(-1, keepdims=True)
        av -= m
        q = np.einsum('bad,bad->ba', av, av)
        s = np.sqrt(q / (D - 1)) + EPS
        av /= s[:, :, None]
        if n1_g is not None:
            av *= n1_g
            av += n1_b
    return av


# --------------------------------------------------------------- model ----
def kernel(input_sequence, output_sequence, emb_in, emb_out, enc_key_W,
           enc_Wq, enc_bq, enc_Wk, enc_bk, n1_g, n1_b, dec_key_W,
           rdr_Wq, rdr_bq, rdr_Wk, rdr_bk, rdr_Wv, rdr_bv,
           dat_Wq, dat_bq, dat_Wk, dat_bk, n2_g, n2_b, n3_g, n3_b,
           voc_W, voc_b):
    f = np.float32
    emb_in = np.asarray(emb_in, f)
    scale = f(1.0 / math.sqrt(D))
    sqrtD = f(math.sqrt(D))
    idx = np.asarray(input_sequence)
    n1_g = np.asarray(n1_g, f)
    n1_b = np.asarray(n1_b, f)

    # -- encoder phase 1: z-trajectory (independent of av) --
    x_enc = emb_in[idx] * sqrtD                                # [B,S,D]
    Z = np.empty((S_ENC, B, D), f)
    z = np.zeros((B, D), f)
    for t in range(S_ENC):
        z = z + x_enc[:, t]
        m = z.mean(-1, keepdims=True)
        z -= m
        q = np.einsum('bd,bd->b', z, z)
        sd = np.sqrt(q / (D - 1)) + EPS
        z /= sd[:, None]
        if n1_g is not None:
            z *= n1_g
            z += n1_b
        Z[t] = z

    # -- encoder phase 2: batched gates (collapsed GEMM) --
    Qa = enc_key_W @ enc_Wq.T + enc_bq                         # [A,D]
    W2 = (enc_Wk.T @ Qa.T).astype(f)                           # [D,A]
    c2 = (enc_bk @ Qa.T).astype(f)                             # [A]
    G_all = _sigmoid((Z.reshape(-1, D) @ W2 + c2) * scale).reshape(
        S_ENC, B, A)

    # -- encoder phase 3: anchor-value recurrence on the NeuronCores --
    # device kernel computes plain LN; apply affine n1_g/n1_b after if
    # they are not identity (setup uses ones/zeros).
    affine = not (np.allclose(n1_g, 1.0) and np.allclose(n1_b, 0.0))
    if affine:
        av = _phase3_host(Z, G_all, n1_g, n1_b)
    else:
        try:
            av = _phase3_on_trn(Z, G_all)
        except Exception:
            av = _phase3_host(Z, G_all, None, None)

    # -- decoder (avx carry is dead code; z path only) --
    Kr = av @ rdr_Wk.T + rdr_bk                                # [B,A,D]
    Vr = av @ rdr_Wv.T + rdr_bv
    # fold the q-projection into the attention: s = zd @ M[b] + c[b]
    M = np.einsum('ed,bae->bda', np.asarray(rdr_Wq, f), Kr)    # [B,D,A]
    c = np.einsum('e,bae->ba', np.asarray(rdr_bq, f), Kr)      # [B,A]
    n2_g = np.asarray(n2_g, f)
    n2_b = np.asarray(n2_b, f)
    zd = Z[-1].copy()                                          # [B,D]
    for t in range(S_DEC):
        a = (np.einsum('bd,bda->ba', zd, M) + c) * scale       # [B,A]
        a -= a.max(axis=-1, keepdims=True)
        e = np.exp(a)
        e /= e.sum(axis=-1, keepdims=True)
        zd = zd + np.einsum('ba,bad->bd', e, Vr)
        m = zd.mean(-1, keepdims=True)
        zd -= m
        q = np.einsum('bd,bd->b', zd, zd)
        sd = np.sqrt(q / (D - 1)) + EPS
        zd /= sd[:, None]
        zd *= n2_g
        zd += n2_b

    # -- logits + log_softmax on host --
    zfin = zd.astype(f)                                        # [B,D]
    logits = zfin @ np.asarray(voc_W, f).T + voc_b             # [B,V]
    logits = logits[:, None, :]
    mx = logits.max(axis=-1, keepdims=True)
    lse = np.log(np.exp(logits - mx).sum(axis=-1, keepdims=True)) + mx
    return (logits - lse).astype(f)


# ------------------------------------------------------------- profile ----
def _profile():
    """Best-available timing of the bass kernel: HW NTFF if possible,
    else CoreSim cost-model time. Returns (exec_ns, source)."""
    nc = _get_nc()
    rng = np.random.default_rng(0)
    Z = rng.standard_normal((S_ENC, B, D)).astype(np.float32)
    G = (0.5 + 0.01 * rng.standard_normal((S_ENC, B, A))).astype(np.float32)
    in_maps = _pack_inputs(Z, G)
    try:
        from concourse.bass_utils import run_bass_kernel_spmd
        res = run_bass_kernel_spmd(nc, in_maps,
                                   core_ids=list(range(N_CORES)), trace=True)
        if res.exec_time_ns:
            return res.exec_time_ns, "hw-ntff"
    except Exception:
        pass
    from concourse.bass_interp import CoreSim
    sim = CoreSim(nc, publish_trace=False)
    for k, v in in_maps[0].items():
        sim.tensor(k)[:] = v
    sim.simulate()
    return int(sim.time), "coresim"
